# revision 38
# baseline (speedup 1.0000x reference)
"""Trainium2 Bass kernel for ModalityAwareDualAttention (dense_cnn).

Sharding: pure data-parallel over batch (32 -> 4 per core x 8 cores).

v2 restructure (DMA- and engine-balanced):
  - host pre-pools xd (2x2 sum) and per-part spatial sums xs; ships both
  - x residual path and output in bf16, packed part-major layouts
  - all per-part weights packed for large-line DMAs (8-16KB per partition)
  - all small bias vectors + per-core gate scalars in ONE [128,110] tile/part
  - SE gate hoisted to once per part (4-col streams, halves LDWEIGHTS)
  - blend fused to 2 passes; elementwise spread over scalar/vector/gpsimd
Algebraic folds identical to v1 (depthwise+pool 0.25 into Wq/Wk/Wv, v-bias
through softmax into upsample bias + fc1 bias, bilinear upsample + gamma as
ktd matmul with mean column, SE+modality gates as per-channel affine).
"""

import numpy as np
import ml_dtypes

import concourse.bass as bass
import concourse.tile as tile
import concourse.mybir as mybir

F32 = mybir.dt.float32
BF16 = mybir.dt.bfloat16
F8 = mybir.dt.float8e4
AF = mybir.ActivationFunctionType
ALU = mybir.AluOpType
DR = mybir.MatmulPerfMode.DoubleRow
KP = 8                     # kc-pair count for DoubleRow (KC // 2)

N_CORES = 8
B, C, H, W, P = 32, 2048, 48, 24, 3
BL = B // N_CORES          # 4 local batches per core
IC = 128                   # q/k inter channels
C4 = 512                   # SE bottleneck
PH = H // P                # 16
HD, WD = PH // 2, W // 2   # 8, 12
N = HD * WD                # 96 attention tokens
HWP = PH * W               # 384 spatial positions per part
KC = C // 128              # 16 channel tiles
NPAIR = 2 * N              # 192
NB = 110                   # bias tile columns

# bias tile column map
BQ, BK = 0, 1
BVBG = 2            # +kc
BB2 = 18            # +kc
BB1 = 34            # +m
BMWC = 38           # +b
BMW = 42            # +b
BXS = 46            # +kc*4+b  (pre-scaled: mean of xp over part)


def _up_matrix(n):
    """[2n, n] bilinear x2 upsample (align_corners=False, edge clamp)."""
    M = np.zeros((2 * n, n), np.float64)
    for o in range(2 * n):
        src = (o + 0.5) / 2.0 - 0.5
        i0 = int(np.floor(src))
        f = src - i0
        M[o, min(max(i0, 0), n - 1)] += 1.0 - f
        M[o, min(max(i0 + 1, 0), n - 1)] += f
    return M


def k_bilinear():
    """[384, 96] upsample matrix: flat(16,24) <- flat(8,12)."""
    return np.kron(_up_matrix(HD), _up_matrix(WD))


def split_excess_waits(nc, max_waits=1):
    """This walrus build rejects multi-sem-wait instructions on some opcodes;
    hoist extra waits onto preceding same-engine no-ops."""
    for f in nc.m.functions:
        for bb in f.blocks:
            insts = bb.instructions
            i = 0
            while i < len(insts):
                ins = insts[i]
                si = ins.sync_info
                if si is not None and si.on_wait and len(si.on_wait) > max_waits:
                    waits = list(si.on_wait)
                    extra, keep = waits[:-max_waits], waits[-max_waits:]
                    nops = []
                    for s in range(0, len(extra), max_waits):
                        nops.append(mybir.InstNoOp(
                            name=nc.get_next_instruction_name(),
                            engine=ins.engine, ins=[], outs=[],
                            sync_info=mybir.SyncInfo(
                                on_wait=extra[s:s + max_waits], on_update=[]),
                        ))
                    ins.sync_info = mybir.SyncInfo(
                        on_wait=keep, on_update=list(si.on_update or []))
                    insts[i:i] = nops
                    i += len(nops)
                i += 1


def build_program(split_waits=True):
    from contextlib import ExitStack
    nc = bass.Bass()

    xbp = nc.dram_tensor("xbp", [P, KC, 128, BL, HWP], BF16, kind="ExternalInput")
    xd8 = nc.dram_tensor("xd8", [P, 128, KP * 2 * BL * N], F8,
                         kind="ExternalInput")
    wv8 = nc.dram_tensor("wv8", [P, 128, KP * 2 * C], F8, kind="ExternalInput")
    wqk8 = nc.dram_tensor("wqk8", [P, 128, KP * 2 * 256], F8,
                          kind="ExternalInput")
    fc1p = nc.dram_tensor("fc1p", [P, 128, KC * C4], BF16, kind="ExternalInput")
    fc2p = nc.dram_tensor("fc2p", [P, 128, 4 * C], BF16, kind="ExternalInput")
    ktb = nc.dram_tensor("ktb", [P, N, HWP + 1], BF16, kind="ExternalInput")
    biasp = nc.dram_tensor("biasp", [P, 128, NB], F32, kind="ExternalInput")
    outp = nc.dram_tensor("outp", [P, KC, 128, BL, HWP], BF16,
                          kind="ExternalOutput")

    with ExitStack() as ctx:
        tc = ctx.enter_context(tile.TileContext(nc))
        pool = lambda name, bufs, **kw: ctx.enter_context(
            tc.tile_pool(name=name, bufs=bufs, **kw))
        wv_pool = pool("wv", 2)
        wqk_pool = pool("wqk", 2)
        fc1_pool = pool("fc1", 1)
        fc2_pool = pool("fc2", 1)
        kt_pool = pool("ktp", 2)
        bias_pool = pool("bias", 2)
        xd_pool = pool("xd", 2)
        xb_pool = pool("xb", 4)
        qk_pool = pool("qk", 4)
        attn_pool = pool("attn", 4)
        g_pool = pool("gg", 2)
        vt_pool = pool("vt", 2)
        ups_pool = pool("ups", 66)
        g2_pool = pool("g2", 18)
        sm_pool = pool("sm", 8)
        se_pool = pool("se", 12)
        cw_pool = pool("cw", 36)
        fin_pool = pool("fin", 8)
        fo_pool = pool("fo", 2)
        ps_vt = pool("ps_vt", 2, space="PSUM")
        ps_bank = pool("ps_bank", 4, space="PSUM")

        # engine helpers for PSUM->SBUF evac with per-partition bias add
        def evac_bias(eng, dst, src, bcol):
            if eng == 0:
                nc.scalar.activation(dst, src, AF.Identity, bias=bcol)
            elif eng == 1:
                nc.vector.tensor_scalar(dst, src, bcol, None, ALU.add)
            else:
                nc.gpsimd.tensor_scalar(dst, src, bcol, None, ALU.add)

        def evac_copy(eng, dst, src):
            if eng == 0:
                nc.scalar.activation(dst, src, AF.Copy)
            elif eng == 1:
                nc.vector.tensor_copy(dst, src)
            else:
                nc.gpsimd.tensor_copy(dst, src)

        def scale1(eng, dst, src, scol):
            # dst = src * scol
            if eng == 0:
                nc.scalar.activation(dst, src, AF.Copy, scale=scol)
            elif eng == 1:
                nc.vector.tensor_scalar(dst, src, scol, None, ALU.mult)
            else:
                nc.gpsimd.tensor_scalar(dst, src, scol, None, ALU.mult)

        def load_part_weights(p):
            """Emit qk/kt/bias/xd/wv loads for part p (SP queue)."""
            w = {}
            wqk_t = wqk_pool.tile([128, KP * 2 * 256], F8, tag="wqk",
                                  name=f"wqk_{p}")
            nc.sync.dma_start(wqk_t[:], wqk8.ap()[p])
            kt_t = kt_pool.tile([N, HWP + 1], BF16, tag="kt", name=f"kt_{p}")
            nc.sync.dma_start(kt_t[:], ktb.ap()[p])
            bias_t = bias_pool.tile([128, NB], F32, tag="bias",
                                    name=f"bias_{p}")
            nc.sync.dma_start(bias_t[:], biasp.ap()[p])
            xd_t = xd_pool.tile([128, KP * 2 * BL * N], F8, tag="xd",
                                name=f"xd_{p}")
            nc.sync.dma_start(xd_t[:], xd8.ap()[p])
            wv_t = wv_pool.tile([128, KP * 2 * C], F8, tag="wv",
                                name=f"wv_{p}")
            nc.sync.dma_start(wv_t[:], wv8.ap()[p])
            w.update(wqk_t=wqk_t, kt_t=kt_t, bias_t=bias_t, xd_t=xd_t,
                     wv_t=wv_t)
            return w

        def load_part_fc(p):
            fc1_t = fc1_pool.tile([128, KC * C4], BF16, tag="fc1",
                                  name=f"fc1_{p}")
            nc.sync.dma_start(fc1_t[:], fc1p.ap()[p])
            fc2_t = fc2_pool.tile([128, 4 * C], BF16, tag="fc2",
                                  name=f"fc2_{p}")
            nc.sync.dma_start(fc2_t[:], fc2p.ap()[p])
            return fc1_t, fc2_t

        w_next = load_part_weights(0)
        fc_next = load_part_fc(0)
        for p in range(P):
            wts = w_next
            wqk_t, kt_t, bias_t = wts["wqk_t"], wts["kt_t"], wts["bias_t"]
            xd_t, wv_t = wts["xd_t"], wts["wv_t"]
            if p > 0:
                fc_next = load_part_fc(p)
            fc1_t, fc2_t = fc_next

            upt = {}
            gap2 = []
            for kc in range(KC):
                gap2.append(g2_pool.tile([128, BL], BF16, tag="g2",
                                         name=f"g2_{p}_{kc}"))

            # DoubleRow operand views: [128, 2, *]
            xdv = [xd_t[:, kp * 2 * BL * N:(kp + 1) * 2 * BL * N]
                   .rearrange("q (two c) -> q two c", two=2)
                   for kp in range(KP)]
            wvv = [wv_t[:, kp * 2 * C:(kp + 1) * 2 * C]
                   .rearrange("q (two c) -> q two c", two=2)
                   for kp in range(KP)]
            qkv = [wqk_t[:, kp * 512:(kp + 1) * 512]
                   .rearrange("q (two c) -> q two c", two=2)
                   for kp in range(KP)]

            for pr in range(BL // 2):
                cols = slice(pr * NPAIR, (pr + 1) * NPAIR)
                # ---------- q/k projections (pair-batched, fp8 DR) ----------
                q_ps = ps_bank.tile([IC, NPAIR], F32, tag="bank")
                for kp in range(KP):
                    nc.tensor.matmul(
                        q_ps[:], qkv[kp][:, :, 0:128], xdv[kp][:, :, cols],
                        start=(kp == 0), stop=(kp == KP - 1), perf_mode=DR)
                q_sb = qk_pool.tile([IC, NPAIR], BF16, tag="qk")
                nc.scalar.activation(q_sb[:], q_ps[:], AF.Identity,
                                     bias=bias_t[:, BQ:BQ + 1])
                k_ps = ps_bank.tile([IC, NPAIR], F32, tag="bank")
                for kp in range(KP):
                    nc.tensor.matmul(
                        k_ps[:], qkv[kp][:, :, 128:256], xdv[kp][:, :, cols],
                        start=(kp == 0), stop=(kp == KP - 1), perf_mode=DR)
                k_sb = qk_pool.tile([IC, NPAIR], BF16, tag="qk")
                nc.scalar.activation(k_sb[:], k_ps[:], AF.Identity,
                                     bias=bias_t[:, BK:BK + 1])

                # ---------- energy + softmax (exps adjacent) ----------
                e_ps = []
                for j in range(2):
                    e = ps_bank.tile([N, N], F32, tag="bank")
                    nc.tensor.matmul(e[:], q_sb[:, j * N:(j + 1) * N],
                                     k_sb[:, j * N:(j + 1) * N],
                                     start=True, stop=True)
                    e_ps.append(e)
                attn_n = []
                ssums = []
                for j in range(2):
                    attn_e = attn_pool.tile([N, N], BF16, tag="attn")
                    s_sum = sm_pool.tile([N, 1], F32, tag="sm")
                    nc.scalar.activation(attn_e[:], e_ps[j][:], AF.Exp,
                                         accum_out=s_sum[:])
                    ssums.append((attn_e, s_sum))
                for j in range(2):
                    attn_e, s_sum = ssums[j]
                    r_sum = sm_pool.tile([N, 1], F32, tag="sm")
                    nc.vector.reciprocal(r_sum[:], s_sum[:])
                    an = attn_pool.tile([N, N], BF16, tag="attn")
                    nc.vector.tensor_scalar(an[:], attn_e[:], r_sum[:],
                                            None, ALU.mult)
                    attn_n.append(an)

                for j in range(2):
                    b = 2 * pr + j
                    # ---------- vT = xd_b^T @ WvT  [N, C], fp8 DR, in two
                    # PSUM ping-pong halves so evac overlaps the next half ---
                    vt_sb = vt_pool.tile([N, C], BF16, tag="vt")
                    for h in range(2):
                        vt_ps = ps_vt.tile([N, C // 2], F32, tag="vt",
                                           name=f"vtps_{p}_{b}_{h}")
                        for kp in range(KP):
                            xdb = xdv[kp][:, :, b * N:(b + 1) * N]
                            for bk in range(2):
                                gc = h * 2 + bk
                                nc.tensor.matmul(
                                    vt_ps[:, bk * 512:(bk + 1) * 512], xdb,
                                    wvv[kp][:, :, gc * 512:(gc + 1) * 512],
                                    start=(kp == 0), stop=(kp == KP - 1),
                                    perf_mode=DR)
                        for bk in range(2):
                            evac_copy((0, 1)[bk],
                                      vt_sb[:, (h * 2 + bk) * 512:
                                            (h * 2 + bk + 1) * 512],
                                      vt_ps[:, bk * 512:(bk + 1) * 512])
                    # ---------- G = attn_n @ KT  [N, 385] ----------
                    g_ps = ps_bank.tile([N, HWP + 1], F32, tag="bank")
                    nc.tensor.matmul(g_ps[:], attn_n[j][:], kt_t[:],
                                     start=True, stop=True)
                    g_sb = g_pool.tile([N, HWP + 1], BF16, tag="g")
                    nc.scalar.activation(g_sb[:], g_ps[:], AF.Copy)
                    # ---------- up chunks + gap ----------
                    for kc in range(KC):
                        up_ps = ps_bank.tile([128, HWP + 1], F32, tag="bank")
                        nc.tensor.matmul(
                            up_ps[:], vt_sb[:, kc * 128:(kc + 1) * 128],
                            g_sb[:], start=True, stop=True)
                        ut = ups_pool.tile([128, HWP], BF16, tag="ups")
                        evac_bias((0, 0, 0, 1)[kc % 4], ut[:], up_ps[:, 0:HWP],
                                  bias_t[:, BVBG + kc:BVBG + kc + 1])
                        upt[(kc, b)] = ut
                        xcol = BXS + kc * 4 + b
                        nc.vector.scalar_tensor_tensor(
                            gap2[kc][:, b:b + 1], bias_t[:, xcol:xcol + 1],
                            1.0, up_ps[:, HWP:HWP + 1], ALU.mult, ALU.add)

            # prefetch next part's weights (frees become available as the
            # last attention ops of this part retire)
            if p + 1 < P:
                w_next = load_part_weights(p + 1)

            # ---------- SE gate, once per part (4 cols); m-outer so only
            # one PSUM buf is held at a time (next part's attention can
            # claim the others) ----------
            h1_t = []
            for m in range(4):
                h_ps = ps_bank.tile([128, BL], F32, tag="bank",
                                    name=f"hps_{p}_{m}")
                for kc in range(KC):
                    nc.tensor.matmul(
                        h_ps[:],
                        fc1_t[:, kc * C4 + m * 128:kc * C4 + (m + 1) * 128],
                        gap2[kc][:], start=(kc == 0), stop=(kc == KC - 1))
                hb = se_pool.tile([128, BL], BF16, tag="se")
                nc.scalar.activation(hb[:], h_ps[:], AF.Relu,
                                     bias=bias_t[:, BB1 + m:BB1 + m + 1])
                h1_t.append(hb)
            cw12 = []
            for kc in range(KC):
                c_ps = ps_bank.tile([128, BL], F32, tag="bank")
                for m in range(4):
                    nc.tensor.matmul(
                        c_ps[:], fc2_t[:, m * C + kc * 128:m * C + (kc + 1) * 128],
                        h1_t[m][:], start=(m == 0), stop=(m == 3))
                cw = se_pool.tile([128, BL], F32, tag="se")
                nc.scalar.activation(cw[:], c_ps[:], AF.Sigmoid,
                                     bias=bias_t[:, BB2 + kc:BB2 + kc + 1])
                tmp = se_pool.tile([128, BL], F32, tag="se")
                nc.vector.tensor_tensor(tmp[:], cw[:], bias_t[:, BMWC:BMWC + 4],
                                        ALU.mult)
                cw1 = cw_pool.tile([128, BL], F32, tag="cw")
                nc.vector.tensor_scalar(cw1[:], tmp[:], 1.0, None, ALU.add)
                cw2 = cw_pool.tile([128, BL], F32, tag="cw")
                nc.vector.tensor_tensor(cw2[:], tmp[:], bias_t[:, BMW:BMW + 4],
                                        ALU.add)
                cw12.append((cw1, cw2))

            # ---------- final blend + store (quad tiles: all 4 batches) ----
            for kc in range(KC):
                cw1, cw2 = cw12[kc]
                xt = xb_pool.tile([128, BL * HWP], BF16, tag="xb")
                nc.sync.dma_start(
                    xt[:], xbp.ap()[p, kc].rearrange("q b s -> q (b s)"))
                fo = fo_pool.tile([128, BL * HWP], BF16, tag="fo")
                for b in range(BL):
                    bs = slice(b * HWP, (b + 1) * HWP)
                    r1 = fin_pool.tile([128, HWP], BF16, tag="fin")
                    if (kc + b) % 2 == 0:
                        nc.scalar.activation(r1[:], xt[:, bs], AF.Copy,
                                             scale=cw1[:, b:b + 1])
                    else:
                        xb_ap, cw1b = bass.broadcast_tensor_aps(
                            xt[:, bs], cw1[:, b:b + 1])
                        nc.gpsimd.tensor_tensor(r1[:], xb_ap, cw1b, ALU.mult)
                    if (kc + b) % 4 != 2:
                        nc.vector.scalar_tensor_tensor(
                            fo[:, bs], upt[(kc, b)][:], cw2[:, b:b + 1], r1[:],
                            ALU.mult, ALU.add)
                    else:
                        # every 4th blend on gpsimd (2-op) to parallelize the
                        # final-part tail and relieve the DVE queue
                        t2 = fin_pool.tile([128, HWP], BF16, tag="fin")
                        u_ap, cw2b = bass.broadcast_tensor_aps(
                            upt[(kc, b)][:], cw2[:, b:b + 1])
                        nc.gpsimd.tensor_tensor(t2[:], u_ap, cw2b, ALU.mult)
                        nc.gpsimd.tensor_tensor(fo[:, bs], t2[:], r1[:],
                                                ALU.add)
                nc.sync.dma_start(
                    outp.ap()[p, kc].rearrange("q b s -> q (b s)"), fo[:])

    if split_waits:
        split_excess_waits(nc)
    return nc


# ---------------------------------------------------------------------------
# Host side
# ---------------------------------------------------------------------------

def _sigmoid(v):
    return 1.0 / (1.0 + np.exp(-v))


def _bf(a):
    return np.ascontiguousarray(a.astype(ml_dtypes.bfloat16))


def _f32(a):
    return np.ascontiguousarray(np.asarray(a, dtype=np.float32))


def prepare_host_inputs(inputs):
    """Fold/transpose weights; returns per-core input dicts."""
    g = {k: np.asarray(v) for k, v in inputs.items()}
    x = _f32(g["x"])

    # modality gate on host (tiny): mw [B, P]
    mf = g["modality"].astype(np.float64)[:, None]
    g1 = np.maximum(mf @ g["gate_w1"].astype(np.float64).T
                    + g["gate_b1"].astype(np.float64), 0.0)
    mw = _sigmoid(g1 @ g["gate_w2"].astype(np.float64).T
                  + g["gate_b2"].astype(np.float64))      # [B, P]

    paq = g["pa_q_w"].astype(np.float64)    # [P, IC, C]
    pak = g["pa_k_w"].astype(np.float64)
    pav = g["pa_v_w"].astype(np.float64)    # [P, C, C]
    dwq_w = g["pa_dw_q_w"].astype(np.float64)   # [P, C]
    dwq_b = g["pa_dw_q_b"].astype(np.float64)
    dwk_w = g["pa_dw_k_w"].astype(np.float64)
    dwk_b = g["pa_dw_k_b"].astype(np.float64)
    gam = g["pa_gamma"].astype(np.float64)      # [P]
    cgam = g["ca_gamma"].astype(np.float64)

    wqT = np.stack([(paq[p] * dwq_w[p][None, :] * 0.25).T for p in range(P)])
    wkT = np.stack([(pak[p] * dwk_w[p][None, :] * 0.25).T for p in range(P)])
    qb = np.stack([g["pa_q_b"][p] + paq[p] @ dwq_b[p] for p in range(P)])
    kb = np.stack([g["pa_k_b"][p] + pak[p] @ dwk_b[p] for p in range(P)])
    wvT = np.stack([0.25 * pav[p].T for p in range(P)])        # [P, C, C]
    vbg = np.stack([gam[p] * g["pa_v_b"][p] for p in range(P)])  # [P, C]

    kb_mat = k_bilinear()                     # [384, 96]
    ktd = np.stack([
        gam[p] * np.concatenate(
            [kb_mat.T, kb_mat.mean(axis=0)[:, None]], axis=1)  # [96, 385]
        for p in range(P)])

    fc1 = g["ca_fc1_w"].astype(np.float64)    # [P, C4, C]
    fc2 = g["ca_fc2_w"].astype(np.float64)    # [P, C, C4]
    b1 = np.stack([g["ca_fc1_b"][p]
                   + fc1[p] @ (gam[p] * g["pa_v_b"][p].astype(np.float64))
                   for p in range(P)])
    b2 = g["ca_fc2_b"].astype(np.float64)

    F8NP = ml_dtypes.float8_e4m3

    # packed fp8 DoubleRow weights
    # wqk8 [P, 128, KP, 2, 256]: [p, i, kp, t, 0:128]=wq, [128:256]=wk
    wqk8 = np.empty((P, 128, KP, 2, 256), np.float32)
    wqk8[..., 0:128] = wqT.reshape(P, KP, 2, 128, IC).transpose(0, 3, 1, 2, 4)
    wqk8[..., 128:256] = wkT.reshape(P, KP, 2, 128, IC).transpose(0, 3, 1, 2, 4)
    wqk8 = np.ascontiguousarray(
        wqk8.reshape(P, 128, KP * 2 * 256).astype(F8NP))
    # wv8 [P, 128, KP, 2, C]
    wv8 = np.ascontiguousarray(
        wvT.reshape(P, KP, 2, 128, C).transpose(0, 3, 1, 2, 4)
        .reshape(P, 128, KP * 2 * C).astype(F8NP))
    # fc1p [P, 128, KC, C4]: fc1T chunks over c
    fc1T = np.stack([fc1[p].T for p in range(P)])   # [P, C, C4]
    fc1p = np.ascontiguousarray(
        fc1T.reshape(P, KC, 128, C4).transpose(0, 2, 1, 3)
    ).reshape(P, 128, KC * C4)
    # fc2p [P, 128, 4, C]: fc2T chunks over d
    fc2T = np.stack([fc2[p].T for p in range(P)])   # [P, C4, C]
    fc2p = np.ascontiguousarray(
        fc2T.reshape(P, 4, 128, C).transpose(0, 2, 1, 3)
    ).reshape(P, 128, 4 * C)

    # x-derived tensors
    # xv [B, KC, 128, P, PH, W]
    xv = x.reshape(B, KC, 128, P, PH, W)
    # pooled sums xd [B, KC, 128, P, N] (sum over 2x2 block)
    xd = xv.reshape(B, KC, 128, P, HD, 2, WD, 2).sum(axis=(5, 7))
    xd = xd.reshape(B, KC, 128, P, N)
    xs = xd.sum(axis=4)                      # [B, KC, 128, P]
    xsm = xs / float(HWP)                    # mean of xp over part

    # shared (per-core-identical) arrays
    shared = {
        "wv8": wv8,
        "wqk8": wqk8,
        "fc1p": _bf(fc1p),
        "fc2p": _bf(fc2p),
        "ktb": _bf(ktd),
    }

    bias_base = np.zeros((P, 128, NB), np.float32)
    for p in range(P):
        bias_base[p, :, BQ] = qb[p]
        bias_base[p, :, BK] = kb[p]
        bias_base[p, :, BVBG:BVBG + KC] = vbg[p].reshape(KC, 128).T
        bias_base[p, :, BB2:BB2 + KC] = b2[p].reshape(KC, 128).T
        bias_base[p, :, BB1:BB1 + 4] = b1[p].reshape(4, 128).T

    per_core = []
    for cix in range(N_CORES):
        bs = slice(cix * BL, (cix + 1) * BL)
        # xbp [P, KC, 128, BL, HWP]
        xbp = np.ascontiguousarray(
            xv[bs].reshape(BL, KC, 128, P, HWP).transpose(3, 1, 2, 0, 4))
        # xd8 [P, 128, KP, 2, BL, N] fp8 (DoubleRow-interleaved kc pairs)
        xd8 = np.ascontiguousarray(
            xd[bs].reshape(BL, KP, 2, 128, P, N)
            .transpose(4, 3, 1, 2, 0, 5)
            .reshape(P, 128, KP * 2 * BL * N).astype(F8NP))
        bias = bias_base.copy()
        mwl = mw[bs]                          # [BL, P]
        for p in range(P):
            bias[p, :, BMWC:BMWC + BL] = (mwl[:, p] * cgam[p])[None, :]
            bias[p, :, BMW:BMW + BL] = mwl[:, p][None, :]
            # xs cols: 46 + kc*4 + b
            bias[p, :, BXS:BXS + KC * BL] = (
                xsm[bs, :, :, p].transpose(1, 0, 2)      # [KC, BL, 128]
                .reshape(KC * BL, 128).T)
        per_core.append({
            "xbp": _bf(xbp),
            "xd8": xd8,
            "biasp": np.ascontiguousarray(bias),
            **shared,
        })
    return per_core


def finish_host_outputs(outs):
    """outs: list of per-core outp [P, KC, 128, BL, HWP] bf16 -> [B, C, H, W]."""
    res = np.empty((B, C, H, W), np.float32)
    for cix, o in enumerate(outs):
        # [P, KC, 128, BL, HWP] -> [BL, KC, 128, P, PH, W]
        of = np.asarray(o).astype(np.float32)
        of = of.reshape(P, KC, 128, BL, PH, W).transpose(3, 1, 2, 0, 4, 5)
        res[cix * BL:(cix + 1) * BL] = of.reshape(BL, C, H, W)
    return res


_CACHE = {}


def kernel(**inputs):
    from concourse.bass_utils import run_bass_kernel_spmd

    per_core = prepare_host_inputs(inputs)
    if "nc" not in _CACHE:
        _CACHE["nc"] = build_program()
    nc = _CACHE["nc"]
    res = run_bass_kernel_spmd(nc, per_core, list(range(N_CORES)))
    return finish_host_outputs(
        [res.results[c]["outp"] for c in range(N_CORES)])


# revision 39
# speedup vs baseline: 1.0634x; 1.0634x over previous
"""Trainium2 Bass kernel for ModalityAwareDualAttention (dense_cnn).

Sharding: pure data-parallel over batch (32 -> 4 per core x 8 cores).

v2 restructure (DMA- and engine-balanced):
  - host pre-pools xd (2x2 sum) and per-part spatial sums xs; ships both
  - x residual path and output in bf16, packed part-major layouts
  - all per-part weights packed for large-line DMAs (8-16KB per partition)
  - all small bias vectors + per-core gate scalars in ONE [128,110] tile/part
  - SE gate hoisted to once per part (4-col streams, halves LDWEIGHTS)
  - blend fused to 2 passes; elementwise spread over scalar/vector/gpsimd
Algebraic folds identical to v1 (depthwise+pool 0.25 into Wq/Wk/Wv, v-bias
through softmax into upsample bias + fc1 bias, bilinear upsample + gamma as
ktd matmul with mean column, SE+modality gates as per-channel affine).
"""

import numpy as np
import ml_dtypes

import concourse.bass as bass
import concourse.tile as tile
import concourse.mybir as mybir

F32 = mybir.dt.float32
BF16 = mybir.dt.bfloat16
F8 = mybir.dt.float8e4
AF = mybir.ActivationFunctionType
ALU = mybir.AluOpType
DR = mybir.MatmulPerfMode.DoubleRow
KP = 8                     # kc-pair count for DoubleRow (KC // 2)

N_CORES = 8
B, C, H, W, P = 32, 2048, 48, 24, 3
BL = B // N_CORES          # 4 local batches per core
IC = 128                   # q/k inter channels
C4 = 512                   # SE bottleneck
PH = H // P                # 16
HD, WD = PH // 2, W // 2   # 8, 12
N = HD * WD                # 96 attention tokens
HWP = PH * W               # 384 spatial positions per part
KC = C // 128              # 16 channel tiles
NPAIR = 2 * N              # 192
NB = 110                   # bias tile columns

# bias tile column map
BQ, BK = 0, 1
BVBG = 2            # +kc
BB2 = 18            # +kc
BB1 = 34            # +m
BMWC = 38           # +b
BMW = 42            # +b
BXS = 46            # +kc*4+b  (pre-scaled: mean of xp over part)


def _up_matrix(n):
    """[2n, n] bilinear x2 upsample (align_corners=False, edge clamp)."""
    M = np.zeros((2 * n, n), np.float64)
    for o in range(2 * n):
        src = (o + 0.5) / 2.0 - 0.5
        i0 = int(np.floor(src))
        f = src - i0
        M[o, min(max(i0, 0), n - 1)] += 1.0 - f
        M[o, min(max(i0 + 1, 0), n - 1)] += f
    return M


def k_bilinear():
    """[384, 96] upsample matrix: flat(16,24) <- flat(8,12)."""
    return np.kron(_up_matrix(HD), _up_matrix(WD))


def split_excess_waits(nc, max_waits=1):
    """This walrus build rejects multi-sem-wait instructions on some opcodes;
    hoist extra waits onto preceding same-engine no-ops."""
    for f in nc.m.functions:
        for bb in f.blocks:
            insts = bb.instructions
            i = 0
            while i < len(insts):
                ins = insts[i]
                si = ins.sync_info
                if si is not None and si.on_wait and len(si.on_wait) > max_waits:
                    waits = list(si.on_wait)
                    extra, keep = waits[:-max_waits], waits[-max_waits:]
                    nops = []
                    for s in range(0, len(extra), max_waits):
                        nops.append(mybir.InstNoOp(
                            name=nc.get_next_instruction_name(),
                            engine=ins.engine, ins=[], outs=[],
                            sync_info=mybir.SyncInfo(
                                on_wait=extra[s:s + max_waits], on_update=[]),
                        ))
                    ins.sync_info = mybir.SyncInfo(
                        on_wait=keep, on_update=list(si.on_update or []))
                    insts[i:i] = nops
                    i += len(nops)
                i += 1


def build_program(split_waits=True):
    from contextlib import ExitStack
    nc = bass.Bass()

    xbp = nc.dram_tensor("xbp", [P, KC, 128, BL, HWP], BF16, kind="ExternalInput")
    xd8 = nc.dram_tensor("xd8", [P, 128, KP * 2 * BL * N], F8,
                         kind="ExternalInput")
    wv8 = nc.dram_tensor("wv8", [P, 128, KP * 2 * C], F8, kind="ExternalInput")
    wqk8 = nc.dram_tensor("wqk8", [P, 128, KP * 2 * 256], F8,
                          kind="ExternalInput")
    fc1p = nc.dram_tensor("fc1p", [P, 128, KC * C4], BF16, kind="ExternalInput")
    fc2p = nc.dram_tensor("fc2p", [P, 128, 4 * C], BF16, kind="ExternalInput")
    ktb = nc.dram_tensor("ktb", [P, N, HWP + 1], BF16, kind="ExternalInput")
    biasp = nc.dram_tensor("biasp", [P, 128, NB], F32, kind="ExternalInput")
    outp = nc.dram_tensor("outp", [P, KC, 128, BL, HWP], BF16,
                          kind="ExternalOutput")

    with ExitStack() as ctx:
        tc = ctx.enter_context(tile.TileContext(nc))
        pool = lambda name, bufs, **kw: ctx.enter_context(
            tc.tile_pool(name=name, bufs=bufs, **kw))
        wv_pool = pool("wv", 2)
        wqk_pool = pool("wqk", 2)
        fc1_pool = pool("fc1", 1)
        fc2_pool = pool("fc2", 1)
        kt_pool = pool("ktp", 2)
        bias_pool = pool("bias", 2)
        xd_pool = pool("xd", 2)
        xb_pool = pool("xb", 4)
        qk_pool = pool("qk", 4)
        attn_pool = pool("attn", 4)
        g_pool = pool("gg", 2)
        vt_pool = pool("vt", 2)
        ups_pool = pool("ups", 66)
        g2_pool = pool("g2", 18)
        sm_pool = pool("sm", 8)
        se_pool = pool("se", 12)
        cw_pool = pool("cw", 36)
        fin_pool = pool("fin", 8)
        fo_pool = pool("fo", 2)
        ps_vt = pool("ps_vt", 2, space="PSUM")
        ps_bank = pool("ps_bank", 4, space="PSUM")

        # engine helpers for PSUM->SBUF evac with per-partition bias add
        def evac_bias(eng, dst, src, bcol):
            if eng == 0:
                nc.scalar.activation(dst, src, AF.Identity, bias=bcol)
            elif eng == 1:
                nc.vector.tensor_scalar(dst, src, bcol, None, ALU.add)
            else:
                nc.gpsimd.tensor_scalar(dst, src, bcol, None, ALU.add)

        def evac_copy(eng, dst, src):
            if eng == 0:
                nc.scalar.activation(dst, src, AF.Copy)
            elif eng == 1:
                nc.vector.tensor_copy(dst, src)
            else:
                nc.gpsimd.tensor_copy(dst, src)

        def scale1(eng, dst, src, scol):
            # dst = src * scol
            if eng == 0:
                nc.scalar.activation(dst, src, AF.Copy, scale=scol)
            elif eng == 1:
                nc.vector.tensor_scalar(dst, src, scol, None, ALU.mult)
            else:
                nc.gpsimd.tensor_scalar(dst, src, scol, None, ALU.mult)

        def load_part_weights(p):
            """Emit qk/kt/bias/xd/wv loads for part p (SP queue)."""
            w = {}
            wqk_t = wqk_pool.tile([128, KP * 2 * 256], F8, tag="wqk",
                                  name=f"wqk_{p}")
            nc.sync.dma_start(wqk_t[:], wqk8.ap()[p])
            kt_t = kt_pool.tile([N, HWP + 1], BF16, tag="kt", name=f"kt_{p}")
            nc.sync.dma_start(kt_t[:], ktb.ap()[p])
            bias_t = bias_pool.tile([128, NB], F32, tag="bias",
                                    name=f"bias_{p}")
            nc.sync.dma_start(bias_t[:], biasp.ap()[p])
            xd_t = xd_pool.tile([128, KP * 2 * BL * N], F8, tag="xd",
                                name=f"xd_{p}")
            nc.sync.dma_start(xd_t[:], xd8.ap()[p])
            wv_t = wv_pool.tile([128, KP * 2 * C], F8, tag="wv",
                                name=f"wv_{p}")
            nc.sync.dma_start(wv_t[:], wv8.ap()[p])
            w.update(wqk_t=wqk_t, kt_t=kt_t, bias_t=bias_t, xd_t=xd_t,
                     wv_t=wv_t)
            return w

        def load_part_fc(p):
            fc1_t = fc1_pool.tile([128, KC * C4], BF16, tag="fc1",
                                  name=f"fc1_{p}")
            nc.sync.dma_start(fc1_t[:], fc1p.ap()[p])
            fc2_t = fc2_pool.tile([128, 4 * C], BF16, tag="fc2",
                                  name=f"fc2_{p}")
            nc.sync.dma_start(fc2_t[:], fc2p.ap()[p])
            return fc1_t, fc2_t

        w_next = load_part_weights(0)
        fc_next = load_part_fc(0)
        for p in range(P):
            wts = w_next
            wqk_t, kt_t, bias_t = wts["wqk_t"], wts["kt_t"], wts["bias_t"]
            xd_t, wv_t = wts["xd_t"], wts["wv_t"]
            if p > 0:
                fc_next = load_part_fc(p)
            fc1_t, fc2_t = fc_next

            upt = {}
            gap2 = []
            for kc in range(KC):
                gap2.append(g2_pool.tile([128, BL], BF16, tag="g2",
                                         name=f"g2_{p}_{kc}"))

            # DoubleRow operand views: [128, 2, *]
            xdv = [xd_t[:, kp * 2 * BL * N:(kp + 1) * 2 * BL * N]
                   .rearrange("q (two c) -> q two c", two=2)
                   for kp in range(KP)]
            wvv = [wv_t[:, kp * 2 * C:(kp + 1) * 2 * C]
                   .rearrange("q (two c) -> q two c", two=2)
                   for kp in range(KP)]
            qkv = [wqk_t[:, kp * 512:(kp + 1) * 512]
                   .rearrange("q (two c) -> q two c", two=2)
                   for kp in range(KP)]

            for pr in range(BL // 2):
                cols = slice(pr * NPAIR, (pr + 1) * NPAIR)
                # ---------- q/k projections (pair-batched, fp8 DR) ----------
                q_ps = ps_bank.tile([IC, NPAIR], F32, tag="bank")
                for kp in range(KP):
                    nc.tensor.matmul(
                        q_ps[:], qkv[kp][:, :, 0:128], xdv[kp][:, :, cols],
                        start=(kp == 0), stop=(kp == KP - 1), perf_mode=DR)
                q_sb = qk_pool.tile([IC, NPAIR], BF16, tag="qk")
                nc.scalar.activation(q_sb[:], q_ps[:], AF.Identity,
                                     bias=bias_t[:, BQ:BQ + 1])
                k_ps = ps_bank.tile([IC, NPAIR], F32, tag="bank")
                for kp in range(KP):
                    nc.tensor.matmul(
                        k_ps[:], qkv[kp][:, :, 128:256], xdv[kp][:, :, cols],
                        start=(kp == 0), stop=(kp == KP - 1), perf_mode=DR)
                k_sb = qk_pool.tile([IC, NPAIR], BF16, tag="qk")
                nc.scalar.activation(k_sb[:], k_ps[:], AF.Identity,
                                     bias=bias_t[:, BK:BK + 1])

                # ---------- energy + softmax (exps adjacent) ----------
                e_ps = []
                for j in range(2):
                    e = ps_bank.tile([N, N], F32, tag="bank")
                    nc.tensor.matmul(e[:], q_sb[:, j * N:(j + 1) * N],
                                     k_sb[:, j * N:(j + 1) * N],
                                     start=True, stop=True)
                    e_ps.append(e)
                attn_n = []
                ssums = []
                for j in range(2):
                    attn_e = attn_pool.tile([N, N], BF16, tag="attn")
                    s_sum = sm_pool.tile([N, 1], F32, tag="sm")
                    nc.scalar.activation(attn_e[:], e_ps[j][:], AF.Exp,
                                         accum_out=s_sum[:])
                    ssums.append((attn_e, s_sum))
                for j in range(2):
                    attn_e, s_sum = ssums[j]
                    r_sum = sm_pool.tile([N, 1], F32, tag="sm")
                    nc.vector.reciprocal(r_sum[:], s_sum[:])
                    an = attn_pool.tile([N, N], BF16, tag="attn")
                    nc.vector.tensor_scalar(an[:], attn_e[:], r_sum[:],
                                            None, ALU.mult)
                    attn_n.append(an)

                for j in range(2):
                    b = 2 * pr + j
                    # ---------- vT = xd_b^T @ WvT  [N, C], fp8 DR, in two
                    # PSUM ping-pong halves so evac overlaps the next half ---
                    vt_sb = vt_pool.tile([N, C], BF16, tag="vt")
                    for h in range(2):
                        vt_ps = ps_vt.tile([N, C // 2], F32, tag="vt",
                                           name=f"vtps_{p}_{b}_{h}")
                        for kp in range(KP):
                            xdb = xdv[kp][:, :, b * N:(b + 1) * N]
                            for bk in range(2):
                                gc = h * 2 + bk
                                nc.tensor.matmul(
                                    vt_ps[:, bk * 512:(bk + 1) * 512], xdb,
                                    wvv[kp][:, :, gc * 512:(gc + 1) * 512],
                                    start=(kp == 0), stop=(kp == KP - 1),
                                    perf_mode=DR)
                        for bk in range(2):
                            evac_copy((0, 1)[bk],
                                      vt_sb[:, (h * 2 + bk) * 512:
                                            (h * 2 + bk + 1) * 512],
                                      vt_ps[:, bk * 512:(bk + 1) * 512])
                    # ---------- G = attn_n @ KT  [N, 385] ----------
                    g_ps = ps_bank.tile([N, HWP + 1], F32, tag="bank")
                    nc.tensor.matmul(g_ps[:], attn_n[j][:], kt_t[:],
                                     start=True, stop=True)
                    g_sb = g_pool.tile([N, HWP + 1], BF16, tag="g")
                    nc.scalar.activation(g_sb[:], g_ps[:], AF.Copy)
                    # ---------- up chunks + gap ----------
                    for kc in range(KC):
                        up_ps = ps_bank.tile([128, HWP + 1], F32, tag="bank")
                        nc.tensor.matmul(
                            up_ps[:], vt_sb[:, kc * 128:(kc + 1) * 128],
                            g_sb[:], start=True, stop=True)
                        ut = ups_pool.tile([128, HWP], BF16, tag="ups")
                        evac_bias((0, 0, 0, 1)[kc % 4], ut[:], up_ps[:, 0:HWP],
                                  bias_t[:, BVBG + kc:BVBG + kc + 1])
                        upt[(kc, b)] = ut
                        xcol = BXS + kc * 4 + b
                        nc.vector.scalar_tensor_tensor(
                            gap2[kc][:, b:b + 1], bias_t[:, xcol:xcol + 1],
                            1.0, up_ps[:, HWP:HWP + 1], ALU.mult, ALU.add)

            # prefetch next part's weights (frees become available as the
            # last attention ops of this part retire)
            if p + 1 < P:
                w_next = load_part_weights(p + 1)

            # ---------- SE gate, once per part (4 cols); m-outer so only
            # one PSUM buf is held at a time (next part's attention can
            # claim the others) ----------
            h1_t = []
            for m in range(4):
                h_ps = ps_bank.tile([128, BL], F32, tag="bank",
                                    name=f"hps_{p}_{m}")
                for kc in range(KC):
                    nc.tensor.matmul(
                        h_ps[:],
                        fc1_t[:, kc * C4 + m * 128:kc * C4 + (m + 1) * 128],
                        gap2[kc][:], start=(kc == 0), stop=(kc == KC - 1))
                hb = se_pool.tile([128, BL], BF16, tag="se")
                nc.scalar.activation(hb[:], h_ps[:], AF.Relu,
                                     bias=bias_t[:, BB1 + m:BB1 + m + 1])
                h1_t.append(hb)
            cw12 = []
            for kc in range(KC):
                c_ps = ps_bank.tile([128, BL], F32, tag="bank")
                for m in range(4):
                    nc.tensor.matmul(
                        c_ps[:], fc2_t[:, m * C + kc * 128:m * C + (kc + 1) * 128],
                        h1_t[m][:], start=(m == 0), stop=(m == 3))
                cw = se_pool.tile([128, BL], F32, tag="se")
                nc.scalar.activation(cw[:], c_ps[:], AF.Sigmoid,
                                     bias=bias_t[:, BB2 + kc:BB2 + kc + 1])
                tmp = se_pool.tile([128, BL], F32, tag="se")
                nc.vector.tensor_tensor(tmp[:], cw[:], bias_t[:, BMWC:BMWC + 4],
                                        ALU.mult)
                cw1 = cw_pool.tile([128, BL], F32, tag="cw")
                nc.vector.tensor_scalar(cw1[:], tmp[:], 1.0, None, ALU.add)
                cw2 = cw_pool.tile([128, BL], F32, tag="cw")
                nc.vector.tensor_tensor(cw2[:], tmp[:], bias_t[:, BMW:BMW + 4],
                                        ALU.add)
                cw12.append((cw1, cw2))

            # ---------- final blend + store (quad tiles: all 4 batches) ----
            for kc in range(KC):
                cw1, cw2 = cw12[kc]
                xt = xb_pool.tile([128, BL * HWP], BF16, tag="xb")
                nc.sync.dma_start(
                    xt[:], xbp.ap()[p, kc].rearrange("q b s -> q (b s)"))
                fo = fo_pool.tile([128, BL * HWP], BF16, tag="fo")
                for b in range(BL):
                    bs = slice(b * HWP, (b + 1) * HWP)
                    r1 = fin_pool.tile([128, HWP], BF16, tag="fin")
                    if (kc + b) % 2 == 0:
                        nc.scalar.activation(r1[:], xt[:, bs], AF.Copy,
                                             scale=cw1[:, b:b + 1])
                    else:
                        xb_ap, cw1b = bass.broadcast_tensor_aps(
                            xt[:, bs], cw1[:, b:b + 1])
                        nc.gpsimd.tensor_tensor(r1[:], xb_ap, cw1b, ALU.mult)
                    if p < P - 1 or (kc + b) % 4 != 2:
                        nc.vector.scalar_tensor_tensor(
                            fo[:, bs], upt[(kc, b)][:], cw2[:, b:b + 1], r1[:],
                            ALU.mult, ALU.add)
                    else:
                        # final part only: every 4th blend on gpsimd (2-op)
                        # to parallelize the tail where DVE otherwise
                        # serializes alone
                        t2 = fin_pool.tile([128, HWP], BF16, tag="fin")
                        u_ap, cw2b = bass.broadcast_tensor_aps(
                            upt[(kc, b)][:], cw2[:, b:b + 1])
                        nc.gpsimd.tensor_tensor(t2[:], u_ap, cw2b, ALU.mult)
                        nc.gpsimd.tensor_tensor(fo[:, bs], t2[:], r1[:],
                                                ALU.add)
                nc.sync.dma_start(
                    outp.ap()[p, kc].rearrange("q b s -> q (b s)"), fo[:])

    if split_waits:
        split_excess_waits(nc)
    return nc


# ---------------------------------------------------------------------------
# Host side
# ---------------------------------------------------------------------------

def _sigmoid(v):
    return 1.0 / (1.0 + np.exp(-v))


def _bf(a):
    return np.ascontiguousarray(a.astype(ml_dtypes.bfloat16))


def _f32(a):
    return np.ascontiguousarray(np.asarray(a, dtype=np.float32))


def prepare_host_inputs(inputs):
    """Fold/transpose weights; returns per-core input dicts."""
    g = {k: np.asarray(v) for k, v in inputs.items()}
    x = _f32(g["x"])

    # modality gate on host (tiny): mw [B, P]
    mf = g["modality"].astype(np.float64)[:, None]
    g1 = np.maximum(mf @ g["gate_w1"].astype(np.float64).T
                    + g["gate_b1"].astype(np.float64), 0.0)
    mw = _sigmoid(g1 @ g["gate_w2"].astype(np.float64).T
                  + g["gate_b2"].astype(np.float64))      # [B, P]

    paq = g["pa_q_w"].astype(np.float64)    # [P, IC, C]
    pak = g["pa_k_w"].astype(np.float64)
    pav = g["pa_v_w"].astype(np.float64)    # [P, C, C]
    dwq_w = g["pa_dw_q_w"].astype(np.float64)   # [P, C]
    dwq_b = g["pa_dw_q_b"].astype(np.float64)
    dwk_w = g["pa_dw_k_w"].astype(np.float64)
    dwk_b = g["pa_dw_k_b"].astype(np.float64)
    gam = g["pa_gamma"].astype(np.float64)      # [P]
    cgam = g["ca_gamma"].astype(np.float64)

    wqT = np.stack([(paq[p] * dwq_w[p][None, :] * 0.25).T for p in range(P)])
    wkT = np.stack([(pak[p] * dwk_w[p][None, :] * 0.25).T for p in range(P)])
    qb = np.stack([g["pa_q_b"][p] + paq[p] @ dwq_b[p] for p in range(P)])
    kb = np.stack([g["pa_k_b"][p] + pak[p] @ dwk_b[p] for p in range(P)])
    wvT = np.stack([0.25 * pav[p].T for p in range(P)])        # [P, C, C]
    vbg = np.stack([gam[p] * g["pa_v_b"][p] for p in range(P)])  # [P, C]

    kb_mat = k_bilinear()                     # [384, 96]
    ktd = np.stack([
        gam[p] * np.concatenate(
            [kb_mat.T, kb_mat.mean(axis=0)[:, None]], axis=1)  # [96, 385]
        for p in range(P)])

    fc1 = g["ca_fc1_w"].astype(np.float64)    # [P, C4, C]
    fc2 = g["ca_fc2_w"].astype(np.float64)    # [P, C, C4]
    b1 = np.stack([g["ca_fc1_b"][p]
                   + fc1[p] @ (gam[p] * g["pa_v_b"][p].astype(np.float64))
                   for p in range(P)])
    b2 = g["ca_fc2_b"].astype(np.float64)

    F8NP = ml_dtypes.float8_e4m3

    # packed fp8 DoubleRow weights
    # wqk8 [P, 128, KP, 2, 256]: [p, i, kp, t, 0:128]=wq, [128:256]=wk
    wqk8 = np.empty((P, 128, KP, 2, 256), np.float32)
    wqk8[..., 0:128] = wqT.reshape(P, KP, 2, 128, IC).transpose(0, 3, 1, 2, 4)
    wqk8[..., 128:256] = wkT.reshape(P, KP, 2, 128, IC).transpose(0, 3, 1, 2, 4)
    wqk8 = np.ascontiguousarray(
        wqk8.reshape(P, 128, KP * 2 * 256).astype(F8NP))
    # wv8 [P, 128, KP, 2, C]
    wv8 = np.ascontiguousarray(
        wvT.reshape(P, KP, 2, 128, C).transpose(0, 3, 1, 2, 4)
        .reshape(P, 128, KP * 2 * C).astype(F8NP))
    # fc1p [P, 128, KC, C4]: fc1T chunks over c
    fc1T = np.stack([fc1[p].T for p in range(P)])   # [P, C, C4]
    fc1p = np.ascontiguousarray(
        fc1T.reshape(P, KC, 128, C4).transpose(0, 2, 1, 3)
    ).reshape(P, 128, KC * C4)
    # fc2p [P, 128, 4, C]: fc2T chunks over d
    fc2T = np.stack([fc2[p].T for p in range(P)])   # [P, C4, C]
    fc2p = np.ascontiguousarray(
        fc2T.reshape(P, 4, 128, C).transpose(0, 2, 1, 3)
    ).reshape(P, 128, 4 * C)

    # x-derived tensors
    # xv [B, KC, 128, P, PH, W]
    xv = x.reshape(B, KC, 128, P, PH, W)
    # pooled sums xd [B, KC, 128, P, N] (sum over 2x2 block)
    xd = xv.reshape(B, KC, 128, P, HD, 2, WD, 2).sum(axis=(5, 7))
    xd = xd.reshape(B, KC, 128, P, N)
    xs = xd.sum(axis=4)                      # [B, KC, 128, P]
    xsm = xs / float(HWP)                    # mean of xp over part

    # shared (per-core-identical) arrays
    shared = {
        "wv8": wv8,
        "wqk8": wqk8,
        "fc1p": _bf(fc1p),
        "fc2p": _bf(fc2p),
        "ktb": _bf(ktd),
    }

    bias_base = np.zeros((P, 128, NB), np.float32)
    for p in range(P):
        bias_base[p, :, BQ] = qb[p]
        bias_base[p, :, BK] = kb[p]
        bias_base[p, :, BVBG:BVBG + KC] = vbg[p].reshape(KC, 128).T
        bias_base[p, :, BB2:BB2 + KC] = b2[p].reshape(KC, 128).T
        bias_base[p, :, BB1:BB1 + 4] = b1[p].reshape(4, 128).T

    per_core = []
    for cix in range(N_CORES):
        bs = slice(cix * BL, (cix + 1) * BL)
        # xbp [P, KC, 128, BL, HWP]
        xbp = np.ascontiguousarray(
            xv[bs].reshape(BL, KC, 128, P, HWP).transpose(3, 1, 2, 0, 4))
        # xd8 [P, 128, KP, 2, BL, N] fp8 (DoubleRow-interleaved kc pairs)
        xd8 = np.ascontiguousarray(
            xd[bs].reshape(BL, KP, 2, 128, P, N)
            .transpose(4, 3, 1, 2, 0, 5)
            .reshape(P, 128, KP * 2 * BL * N).astype(F8NP))
        bias = bias_base.copy()
        mwl = mw[bs]                          # [BL, P]
        for p in range(P):
            bias[p, :, BMWC:BMWC + BL] = (mwl[:, p] * cgam[p])[None, :]
            bias[p, :, BMW:BMW + BL] = mwl[:, p][None, :]
            # xs cols: 46 + kc*4 + b
            bias[p, :, BXS:BXS + KC * BL] = (
                xsm[bs, :, :, p].transpose(1, 0, 2)      # [KC, BL, 128]
                .reshape(KC * BL, 128).T)
        per_core.append({
            "xbp": _bf(xbp),
            "xd8": xd8,
            "biasp": np.ascontiguousarray(bias),
            **shared,
        })
    return per_core


def finish_host_outputs(outs):
    """outs: list of per-core outp [P, KC, 128, BL, HWP] bf16 -> [B, C, H, W]."""
    res = np.empty((B, C, H, W), np.float32)
    for cix, o in enumerate(outs):
        # [P, KC, 128, BL, HWP] -> [BL, KC, 128, P, PH, W]
        of = np.asarray(o).astype(np.float32)
        of = of.reshape(P, KC, 128, BL, PH, W).transpose(3, 1, 2, 0, 4, 5)
        res[cix * BL:(cix + 1) * BL] = of.reshape(BL, C, H, W)
    return res


_CACHE = {}


def kernel(**inputs):
    from concourse.bass_utils import run_bass_kernel_spmd

    per_core = prepare_host_inputs(inputs)
    if "nc" not in _CACHE:
        _CACHE["nc"] = build_program()
    nc = _CACHE["nc"]
    res = run_bass_kernel_spmd(nc, per_core, list(range(N_CORES)))
    return finish_host_outputs(
        [res.results[c]["outp"] for c in range(N_CORES)])


# revision 40
# speedup vs baseline: 1.0930x; 1.0278x over previous
"""Trainium2 Bass kernel for ModalityAwareDualAttention (dense_cnn).

Sharding: pure data-parallel over batch (32 -> 4 per core x 8 cores).

v2 restructure (DMA- and engine-balanced):
  - host pre-pools xd (2x2 sum) and per-part spatial sums xs; ships both
  - x residual path and output in bf16, packed part-major layouts
  - all per-part weights packed for large-line DMAs (8-16KB per partition)
  - all small bias vectors + per-core gate scalars in ONE [128,110] tile/part
  - SE gate hoisted to once per part (4-col streams, halves LDWEIGHTS)
  - blend fused to 2 passes; elementwise spread over scalar/vector/gpsimd
Algebraic folds identical to v1 (depthwise+pool 0.25 into Wq/Wk/Wv, v-bias
through softmax into upsample bias + fc1 bias, bilinear upsample + gamma as
ktd matmul with mean column, SE+modality gates as per-channel affine).
"""

import numpy as np
import ml_dtypes

import concourse.bass as bass
import concourse.tile as tile
import concourse.mybir as mybir

F32 = mybir.dt.float32
BF16 = mybir.dt.bfloat16
F8 = mybir.dt.float8e4
AF = mybir.ActivationFunctionType
ALU = mybir.AluOpType
DR = mybir.MatmulPerfMode.DoubleRow
KP = 8                     # kc-pair count for DoubleRow (KC // 2)

N_CORES = 8
B, C, H, W, P = 32, 2048, 48, 24, 3
BL = B // N_CORES          # 4 local batches per core
IC = 128                   # q/k inter channels
C4 = 512                   # SE bottleneck
PH = H // P                # 16
HD, WD = PH // 2, W // 2   # 8, 12
N = HD * WD                # 96 attention tokens
HWP = PH * W               # 384 spatial positions per part
KC = C // 128              # 16 channel tiles
NPAIR = 2 * N              # 192
NB = 110                   # bias tile columns

# bias tile column map
BQ, BK = 0, 1
BVBG = 2            # +kc
BB2 = 18            # +kc
BB1 = 34            # +m
BMWC = 38           # +b
BMW = 42            # +b
BXS = 46            # +kc*4+b  (pre-scaled: mean of xp over part)


def _up_matrix(n):
    """[2n, n] bilinear x2 upsample (align_corners=False, edge clamp)."""
    M = np.zeros((2 * n, n), np.float64)
    for o in range(2 * n):
        src = (o + 0.5) / 2.0 - 0.5
        i0 = int(np.floor(src))
        f = src - i0
        M[o, min(max(i0, 0), n - 1)] += 1.0 - f
        M[o, min(max(i0 + 1, 0), n - 1)] += f
    return M


def k_bilinear():
    """[384, 96] upsample matrix: flat(16,24) <- flat(8,12)."""
    return np.kron(_up_matrix(HD), _up_matrix(WD))


def split_excess_waits(nc, max_waits=1):
    """This walrus build rejects multi-sem-wait instructions on some opcodes;
    hoist extra waits onto preceding same-engine no-ops."""
    for f in nc.m.functions:
        for bb in f.blocks:
            insts = bb.instructions
            i = 0
            while i < len(insts):
                ins = insts[i]
                si = ins.sync_info
                if si is not None and si.on_wait and len(si.on_wait) > max_waits:
                    waits = list(si.on_wait)
                    extra, keep = waits[:-max_waits], waits[-max_waits:]
                    nops = []
                    for s in range(0, len(extra), max_waits):
                        nops.append(mybir.InstNoOp(
                            name=nc.get_next_instruction_name(),
                            engine=ins.engine, ins=[], outs=[],
                            sync_info=mybir.SyncInfo(
                                on_wait=extra[s:s + max_waits], on_update=[]),
                        ))
                    ins.sync_info = mybir.SyncInfo(
                        on_wait=keep, on_update=list(si.on_update or []))
                    insts[i:i] = nops
                    i += len(nops)
                i += 1


def build_program(split_waits=True):
    from contextlib import ExitStack
    nc = bass.Bass()

    xbp = nc.dram_tensor("xbp", [P, KC, 128, BL, HWP], BF16, kind="ExternalInput")
    xd8 = nc.dram_tensor("xd8", [P, 128, KP * 2 * BL * N], F8,
                         kind="ExternalInput")
    wv8 = nc.dram_tensor("wv8", [P, 128, KP * 2 * C], F8, kind="ExternalInput")
    wqk8 = nc.dram_tensor("wqk8", [P, 128, KP * 2 * 256], F8,
                          kind="ExternalInput")
    fc1p = nc.dram_tensor("fc1p", [P, 128, KC * C4], BF16, kind="ExternalInput")
    fc2p = nc.dram_tensor("fc2p", [P, 128, 4 * C], BF16, kind="ExternalInput")
    ktb = nc.dram_tensor("ktb", [P, N, HWP + 1], BF16, kind="ExternalInput")
    biasp = nc.dram_tensor("biasp", [P, 128, NB], F32, kind="ExternalInput")
    outp = nc.dram_tensor("outp", [P, KC, 128, BL, HWP], BF16,
                          kind="ExternalOutput")

    with ExitStack() as ctx:
        tc = ctx.enter_context(tile.TileContext(nc))
        pool = lambda name, bufs, **kw: ctx.enter_context(
            tc.tile_pool(name=name, bufs=bufs, **kw))
        wv_pool = pool("wv", 2)
        wqk_pool = pool("wqk", 2)
        fc1_pool = pool("fc1", 1)
        fc2_pool = pool("fc2", 1)
        kt_pool = pool("ktp", 2)
        bias_pool = pool("bias", 2)
        xd_pool = pool("xd", 2)
        xb_pool = pool("xb", 4)
        qk_pool = pool("qk", 4)
        attn_pool = pool("attn", 4)
        g_pool = pool("gg", 2)
        vt_pool = pool("vt", 2)
        ups_pool = pool("ups", 66)
        g2_pool = pool("g2", 18)
        sm_pool = pool("sm", 8)
        se_pool = pool("se", 12)
        cw_pool = pool("cw", 36)
        fin_pool = pool("fin", 8)
        fo_pool = pool("fo", 2)
        ps_vt = pool("ps_vt", 2, space="PSUM")
        ps_bank = pool("ps_bank", 4, space="PSUM")

        # engine helpers for PSUM->SBUF evac with per-partition bias add
        def evac_bias(eng, dst, src, bcol):
            if eng == 0:
                nc.scalar.activation(dst, src, AF.Identity, bias=bcol)
            elif eng == 1:
                nc.vector.tensor_scalar(dst, src, bcol, None, ALU.add)
            else:
                nc.gpsimd.tensor_scalar(dst, src, bcol, None, ALU.add)

        def evac_copy(eng, dst, src):
            if eng == 0:
                nc.scalar.activation(dst, src, AF.Copy)
            elif eng == 1:
                nc.vector.tensor_copy(dst, src)
            else:
                nc.gpsimd.tensor_copy(dst, src)

        def scale1(eng, dst, src, scol):
            # dst = src * scol
            if eng == 0:
                nc.scalar.activation(dst, src, AF.Copy, scale=scol)
            elif eng == 1:
                nc.vector.tensor_scalar(dst, src, scol, None, ALU.mult)
            else:
                nc.gpsimd.tensor_scalar(dst, src, scol, None, ALU.mult)

        def load_part_weights(p):
            """Emit qk/kt/bias/xd/wv loads for part p (SP queue)."""
            w = {}
            wqk_t = wqk_pool.tile([128, KP * 2 * 256], F8, tag="wqk",
                                  name=f"wqk_{p}")
            nc.sync.dma_start(wqk_t[:], wqk8.ap()[p])
            kt_t = kt_pool.tile([N, HWP + 1], BF16, tag="kt", name=f"kt_{p}")
            nc.sync.dma_start(kt_t[:], ktb.ap()[p])
            bias_t = bias_pool.tile([128, NB], F32, tag="bias",
                                    name=f"bias_{p}")
            nc.sync.dma_start(bias_t[:], biasp.ap()[p])
            xd_t = xd_pool.tile([128, KP * 2 * BL * N], F8, tag="xd",
                                name=f"xd_{p}")
            nc.sync.dma_start(xd_t[:], xd8.ap()[p])
            wv_t = wv_pool.tile([128, KP * 2 * C], F8, tag="wv",
                                name=f"wv_{p}")
            nc.sync.dma_start(wv_t[:], wv8.ap()[p])
            w.update(wqk_t=wqk_t, kt_t=kt_t, bias_t=bias_t, xd_t=xd_t,
                     wv_t=wv_t)
            return w

        def load_part_fc(p):
            fc1_t = fc1_pool.tile([128, KC * C4], BF16, tag="fc1",
                                  name=f"fc1_{p}")
            nc.sync.dma_start(fc1_t[:], fc1p.ap()[p])
            fc2_t = fc2_pool.tile([128, 4 * C], BF16, tag="fc2",
                                  name=f"fc2_{p}")
            nc.sync.dma_start(fc2_t[:], fc2p.ap()[p])
            return fc1_t, fc2_t

        w_next = load_part_weights(0)
        fc_next = load_part_fc(0)
        for p in range(P):
            wts = w_next
            wqk_t, kt_t, bias_t = wts["wqk_t"], wts["kt_t"], wts["bias_t"]
            xd_t, wv_t = wts["xd_t"], wts["wv_t"]
            if p > 0:
                fc_next = load_part_fc(p)
            fc1_t, fc2_t = fc_next

            upt = {}
            gap2 = []
            for kc in range(KC):
                gap2.append(g2_pool.tile([128, BL], BF16, tag="g2",
                                         name=f"g2_{p}_{kc}"))

            # DoubleRow operand views: [128, 2, *]
            xdv = [xd_t[:, kp * 2 * BL * N:(kp + 1) * 2 * BL * N]
                   .rearrange("q (two c) -> q two c", two=2)
                   for kp in range(KP)]
            wvv = [wv_t[:, kp * 2 * C:(kp + 1) * 2 * C]
                   .rearrange("q (two c) -> q two c", two=2)
                   for kp in range(KP)]
            qkv = [wqk_t[:, kp * 512:(kp + 1) * 512]
                   .rearrange("q (two c) -> q two c", two=2)
                   for kp in range(KP)]

            for pr in range(BL // 2):
                cols = slice(pr * NPAIR, (pr + 1) * NPAIR)
                # ---------- q/k projections (pair-batched, fp8 DR) ----------
                q_ps = ps_bank.tile([IC, NPAIR], F32, tag="bank")
                for kp in range(KP):
                    nc.tensor.matmul(
                        q_ps[:], qkv[kp][:, :, 0:128], xdv[kp][:, :, cols],
                        start=(kp == 0), stop=(kp == KP - 1), perf_mode=DR)
                q_sb = qk_pool.tile([IC, NPAIR], BF16, tag="qk")
                nc.scalar.activation(q_sb[:], q_ps[:], AF.Identity,
                                     bias=bias_t[:, BQ:BQ + 1])
                k_ps = ps_bank.tile([IC, NPAIR], F32, tag="bank")
                for kp in range(KP):
                    nc.tensor.matmul(
                        k_ps[:], qkv[kp][:, :, 128:256], xdv[kp][:, :, cols],
                        start=(kp == 0), stop=(kp == KP - 1), perf_mode=DR)
                k_sb = qk_pool.tile([IC, NPAIR], BF16, tag="qk")
                nc.scalar.activation(k_sb[:], k_ps[:], AF.Identity,
                                     bias=bias_t[:, BK:BK + 1])

                # ---------- energy + softmax (exps adjacent) ----------
                e_ps = []
                for j in range(2):
                    e = ps_bank.tile([N, N], F32, tag="bank")
                    nc.tensor.matmul(e[:], q_sb[:, j * N:(j + 1) * N],
                                     k_sb[:, j * N:(j + 1) * N],
                                     start=True, stop=True)
                    e_ps.append(e)
                attn_n = []
                ssums = []
                for j in range(2):
                    attn_e = attn_pool.tile([N, N], BF16, tag="attn")
                    s_sum = sm_pool.tile([N, 1], F32, tag="sm")
                    nc.scalar.activation(attn_e[:], e_ps[j][:], AF.Exp,
                                         accum_out=s_sum[:])
                    ssums.append((attn_e, s_sum))
                for j in range(2):
                    attn_e, s_sum = ssums[j]
                    r_sum = sm_pool.tile([N, 1], F32, tag="sm")
                    nc.vector.reciprocal(r_sum[:], s_sum[:])
                    an = attn_pool.tile([N, N], BF16, tag="attn")
                    nc.vector.tensor_scalar(an[:], attn_e[:], r_sum[:],
                                            None, ALU.mult)
                    attn_n.append(an)

                for j in range(2):
                    b = 2 * pr + j
                    # ---------- vT = xd_b^T @ WvT  [N, C], fp8 DR, in two
                    # PSUM ping-pong halves so evac overlaps the next half ---
                    vt_sb = vt_pool.tile([N, C], BF16, tag="vt")
                    for h in range(2):
                        vt_ps = ps_vt.tile([N, C // 2], F32, tag="vt",
                                           name=f"vtps_{p}_{b}_{h}")
                        for kp in range(KP):
                            xdb = xdv[kp][:, :, b * N:(b + 1) * N]
                            for bk in range(2):
                                gc = h * 2 + bk
                                nc.tensor.matmul(
                                    vt_ps[:, bk * 512:(bk + 1) * 512], xdb,
                                    wvv[kp][:, :, gc * 512:(gc + 1) * 512],
                                    start=(kp == 0), stop=(kp == KP - 1),
                                    perf_mode=DR)
                        for bk in range(2):
                            evac_copy((0, 1)[bk],
                                      vt_sb[:, (h * 2 + bk) * 512:
                                            (h * 2 + bk + 1) * 512],
                                      vt_ps[:, bk * 512:(bk + 1) * 512])
                    # ---------- G = attn_n @ KT  [N, 385] ----------
                    g_ps = ps_bank.tile([N, HWP + 1], F32, tag="bank")
                    nc.tensor.matmul(g_ps[:], attn_n[j][:], kt_t[:],
                                     start=True, stop=True)
                    g_sb = g_pool.tile([N, HWP + 1], BF16, tag="g")
                    nc.scalar.activation(g_sb[:], g_ps[:], AF.Copy)
                    # ---------- up chunks + gap ----------
                    for kc in range(KC):
                        up_ps = ps_bank.tile([128, HWP + 1], F32, tag="bank")
                        nc.tensor.matmul(
                            up_ps[:], vt_sb[:, kc * 128:(kc + 1) * 128],
                            g_sb[:], start=True, stop=True)
                        ut = ups_pool.tile([128, HWP], BF16, tag="ups")
                        evac_bias((0, 0, 0, 1)[kc % 4], ut[:], up_ps[:, 0:HWP],
                                  bias_t[:, BVBG + kc:BVBG + kc + 1])
                        upt[(kc, b)] = ut
                        xcol = BXS + kc * 4 + b
                        nc.vector.scalar_tensor_tensor(
                            gap2[kc][:, b:b + 1], bias_t[:, xcol:xcol + 1],
                            1.0, up_ps[:, HWP:HWP + 1], ALU.mult, ALU.add)

            # prefetch next part's weights (frees become available as the
            # last attention ops of this part retire)
            if p + 1 < P:
                w_next = load_part_weights(p + 1)

            # ---------- SE gate, once per part (4 cols); m-outer so only
            # one PSUM buf is held at a time (next part's attention can
            # claim the others) ----------
            h1_t = []
            for m in range(4):
                h_ps = ps_bank.tile([128, BL], F32, tag="bank",
                                    name=f"hps_{p}_{m}")
                for kc in range(KC):
                    nc.tensor.matmul(
                        h_ps[:],
                        fc1_t[:, kc * C4 + m * 128:kc * C4 + (m + 1) * 128],
                        gap2[kc][:], start=(kc == 0), stop=(kc == KC - 1))
                hb = se_pool.tile([128, BL], BF16, tag="se")
                nc.scalar.activation(hb[:], h_ps[:], AF.Relu,
                                     bias=bias_t[:, BB1 + m:BB1 + m + 1])
                h1_t.append(hb)
            cw12 = []
            for kc in range(KC):
                c_ps = ps_bank.tile([128, BL], F32, tag="bank")
                for m in range(4):
                    nc.tensor.matmul(
                        c_ps[:], fc2_t[:, m * C + kc * 128:m * C + (kc + 1) * 128],
                        h1_t[m][:], start=(m == 0), stop=(m == 3))
                cw = se_pool.tile([128, BL], F32, tag="se")
                nc.scalar.activation(cw[:], c_ps[:], AF.Sigmoid,
                                     bias=bias_t[:, BB2 + kc:BB2 + kc + 1])
                tmp = se_pool.tile([128, BL], F32, tag="se")
                nc.vector.tensor_tensor(tmp[:], cw[:], bias_t[:, BMWC:BMWC + 4],
                                        ALU.mult)
                cw1 = cw_pool.tile([128, BL], F32, tag="cw")
                nc.vector.tensor_scalar(cw1[:], tmp[:], 1.0, None, ALU.add)
                cw2 = cw_pool.tile([128, BL], F32, tag="cw")
                nc.vector.tensor_tensor(cw2[:], tmp[:], bias_t[:, BMW:BMW + 4],
                                        ALU.add)
                cw12.append((cw1, cw2))

            # ---------- final blend + store (quad tiles: all 4 batches) ----
            for kc in range(KC):
                cw1, cw2 = cw12[kc]
                xt = xb_pool.tile([128, BL * HWP], BF16, tag="xb")
                nc.sync.dma_start(
                    xt[:], xbp.ap()[p, kc].rearrange("q b s -> q (b s)"))
                fo = fo_pool.tile([128, BL * HWP], BF16, tag="fo")
                for b in range(BL):
                    bs = slice(b * HWP, (b + 1) * HWP)
                    r1 = fin_pool.tile([128, HWP], BF16, tag="fin")
                    if (kc + b) % 2 == 0:
                        nc.scalar.activation(r1[:], xt[:, bs], AF.Copy,
                                             scale=cw1[:, b:b + 1])
                    else:
                        xb_ap, cw1b = bass.broadcast_tensor_aps(
                            xt[:, bs], cw1[:, b:b + 1])
                        nc.gpsimd.tensor_tensor(r1[:], xb_ap, cw1b, ALU.mult)
                    nc.vector.scalar_tensor_tensor(
                        fo[:, bs], upt[(kc, b)][:], cw2[:, b:b + 1], r1[:],
                        ALU.mult, ALU.add)
                nc.sync.dma_start(
                    outp.ap()[p, kc].rearrange("q b s -> q (b s)"), fo[:])

    if split_waits:
        split_excess_waits(nc)
    return nc


# ---------------------------------------------------------------------------
# Host side
# ---------------------------------------------------------------------------

def _sigmoid(v):
    return 1.0 / (1.0 + np.exp(-v))


def _bf(a):
    return np.ascontiguousarray(a.astype(ml_dtypes.bfloat16))


def _f32(a):
    return np.ascontiguousarray(np.asarray(a, dtype=np.float32))


def prepare_host_inputs(inputs):
    """Fold/transpose weights; returns per-core input dicts."""
    g = {k: np.asarray(v) for k, v in inputs.items()}
    x = _f32(g["x"])

    # modality gate on host (tiny): mw [B, P]
    mf = g["modality"].astype(np.float64)[:, None]
    g1 = np.maximum(mf @ g["gate_w1"].astype(np.float64).T
                    + g["gate_b1"].astype(np.float64), 0.0)
    mw = _sigmoid(g1 @ g["gate_w2"].astype(np.float64).T
                  + g["gate_b2"].astype(np.float64))      # [B, P]

    paq = g["pa_q_w"].astype(np.float64)    # [P, IC, C]
    pak = g["pa_k_w"].astype(np.float64)
    pav = g["pa_v_w"].astype(np.float64)    # [P, C, C]
    dwq_w = g["pa_dw_q_w"].astype(np.float64)   # [P, C]
    dwq_b = g["pa_dw_q_b"].astype(np.float64)
    dwk_w = g["pa_dw_k_w"].astype(np.float64)
    dwk_b = g["pa_dw_k_b"].astype(np.float64)
    gam = g["pa_gamma"].astype(np.float64)      # [P]
    cgam = g["ca_gamma"].astype(np.float64)

    wqT = np.stack([(paq[p] * dwq_w[p][None, :] * 0.25).T for p in range(P)])
    wkT = np.stack([(pak[p] * dwk_w[p][None, :] * 0.25).T for p in range(P)])
    qb = np.stack([g["pa_q_b"][p] + paq[p] @ dwq_b[p] for p in range(P)])
    kb = np.stack([g["pa_k_b"][p] + pak[p] @ dwk_b[p] for p in range(P)])
    wvT = np.stack([0.25 * pav[p].T for p in range(P)])        # [P, C, C]
    vbg = np.stack([gam[p] * g["pa_v_b"][p] for p in range(P)])  # [P, C]

    kb_mat = k_bilinear()                     # [384, 96]
    ktd = np.stack([
        gam[p] * np.concatenate(
            [kb_mat.T, kb_mat.mean(axis=0)[:, None]], axis=1)  # [96, 385]
        for p in range(P)])

    fc1 = g["ca_fc1_w"].astype(np.float64)    # [P, C4, C]
    fc2 = g["ca_fc2_w"].astype(np.float64)    # [P, C, C4]
    b1 = np.stack([g["ca_fc1_b"][p]
                   + fc1[p] @ (gam[p] * g["pa_v_b"][p].astype(np.float64))
                   for p in range(P)])
    b2 = g["ca_fc2_b"].astype(np.float64)

    F8NP = ml_dtypes.float8_e4m3

    # packed fp8 DoubleRow weights
    # wqk8 [P, 128, KP, 2, 256]: [p, i, kp, t, 0:128]=wq, [128:256]=wk
    wqk8 = np.empty((P, 128, KP, 2, 256), np.float32)
    wqk8[..., 0:128] = wqT.reshape(P, KP, 2, 128, IC).transpose(0, 3, 1, 2, 4)
    wqk8[..., 128:256] = wkT.reshape(P, KP, 2, 128, IC).transpose(0, 3, 1, 2, 4)
    wqk8 = np.ascontiguousarray(
        wqk8.reshape(P, 128, KP * 2 * 256).astype(F8NP))
    # wv8 [P, 128, KP, 2, C]
    wv8 = np.ascontiguousarray(
        wvT.reshape(P, KP, 2, 128, C).transpose(0, 3, 1, 2, 4)
        .reshape(P, 128, KP * 2 * C).astype(F8NP))
    # fc1p [P, 128, KC, C4]: fc1T chunks over c
    fc1T = np.stack([fc1[p].T for p in range(P)])   # [P, C, C4]
    fc1p = np.ascontiguousarray(
        fc1T.reshape(P, KC, 128, C4).transpose(0, 2, 1, 3)
    ).reshape(P, 128, KC * C4)
    # fc2p [P, 128, 4, C]: fc2T chunks over d
    fc2T = np.stack([fc2[p].T for p in range(P)])   # [P, C4, C]
    fc2p = np.ascontiguousarray(
        fc2T.reshape(P, 4, 128, C).transpose(0, 2, 1, 3)
    ).reshape(P, 128, 4 * C)

    # x-derived tensors
    # xv [B, KC, 128, P, PH, W]
    xv = x.reshape(B, KC, 128, P, PH, W)
    # pooled sums xd [B, KC, 128, P, N] (sum over 2x2 block)
    xd = xv.reshape(B, KC, 128, P, HD, 2, WD, 2).sum(axis=(5, 7))
    xd = xd.reshape(B, KC, 128, P, N)
    xs = xd.sum(axis=4)                      # [B, KC, 128, P]
    xsm = xs / float(HWP)                    # mean of xp over part

    # shared (per-core-identical) arrays
    shared = {
        "wv8": wv8,
        "wqk8": wqk8,
        "fc1p": _bf(fc1p),
        "fc2p": _bf(fc2p),
        "ktb": _bf(ktd),
    }

    bias_base = np.zeros((P, 128, NB), np.float32)
    for p in range(P):
        bias_base[p, :, BQ] = qb[p]
        bias_base[p, :, BK] = kb[p]
        bias_base[p, :, BVBG:BVBG + KC] = vbg[p].reshape(KC, 128).T
        bias_base[p, :, BB2:BB2 + KC] = b2[p].reshape(KC, 128).T
        bias_base[p, :, BB1:BB1 + 4] = b1[p].reshape(4, 128).T

    per_core = []
    for cix in range(N_CORES):
        bs = slice(cix * BL, (cix + 1) * BL)
        # xbp [P, KC, 128, BL, HWP]
        xbp = np.ascontiguousarray(
            xv[bs].reshape(BL, KC, 128, P, HWP).transpose(3, 1, 2, 0, 4))
        # xd8 [P, 128, KP, 2, BL, N] fp8 (DoubleRow-interleaved kc pairs)
        xd8 = np.ascontiguousarray(
            xd[bs].reshape(BL, KP, 2, 128, P, N)
            .transpose(4, 3, 1, 2, 0, 5)
            .reshape(P, 128, KP * 2 * BL * N).astype(F8NP))
        bias = bias_base.copy()
        mwl = mw[bs]                          # [BL, P]
        for p in range(P):
            bias[p, :, BMWC:BMWC + BL] = (mwl[:, p] * cgam[p])[None, :]
            bias[p, :, BMW:BMW + BL] = mwl[:, p][None, :]
            # xs cols: 46 + kc*4 + b
            bias[p, :, BXS:BXS + KC * BL] = (
                xsm[bs, :, :, p].transpose(1, 0, 2)      # [KC, BL, 128]
                .reshape(KC * BL, 128).T)
        per_core.append({
            "xbp": _bf(xbp),
            "xd8": xd8,
            "biasp": np.ascontiguousarray(bias),
            **shared,
        })
    return per_core


def finish_host_outputs(outs):
    """outs: list of per-core outp [P, KC, 128, BL, HWP] bf16 -> [B, C, H, W]."""
    res = np.empty((B, C, H, W), np.float32)
    for cix, o in enumerate(outs):
        # [P, KC, 128, BL, HWP] -> [BL, KC, 128, P, PH, W]
        of = np.asarray(o).astype(np.float32)
        of = of.reshape(P, KC, 128, BL, PH, W).transpose(3, 1, 2, 0, 4, 5)
        res[cix * BL:(cix + 1) * BL] = of.reshape(BL, C, H, W)
    return res


_CACHE = {}


def kernel(**inputs):
    from concourse.bass_utils import run_bass_kernel_spmd

    per_core = prepare_host_inputs(inputs)
    if "nc" not in _CACHE:
        _CACHE["nc"] = build_program()
    nc = _CACHE["nc"]
    res = run_bass_kernel_spmd(nc, per_core, list(range(N_CORES)))
    return finish_host_outputs(
        [res.results[c]["outp"] for c in range(N_CORES)])
